# revision 24
# baseline (speedup 1.0000x reference)
"""Trainium2 Bass kernel for the UR5e reflected-mass cost function.

Math (per sample n of 131072 = 2048 b x 64 h):
  q = x[b,h,6:12], hand = x[b,h,19:22]
  FK chain (6 DH joints) -> frame origins p_0..p_6, z-axes z_0..z_6
  J[i,j] = z_j x (p_{i+1} - p_j)  (j<=i)        geometric Jacobian columns
  M = sum_i m_i J_i^T J_i + 0.1 I               6x6 SPD mass matrix
  d = hand - p_6 ; vd_j = J[5,j] . d
  s = vd^T M^-1 vd = |L^-1 vd|^2  (M = L L^T Cholesky, forward-solve only)
  cost = |d|^2 / s ;  out[b] = -sum_h cost

Implementation: every per-sample scalar is a [128,128] f32 SBUF tile
(16384 samples per core, 8 cores data-parallel over b).  The whole
computation is built as a symbolic scalar DAG with CSE + constant
folding, then emitted as DVE/ACT instructions balanced across engines
via the Tile framework.
"""

import math
import numpy as np

# ----------------------------------------------------------------------------
# symbolic scalar DAG
# ----------------------------------------------------------------------------

PI = math.pi
DH_A = [0.0, -0.425, -0.3922, 0.0, 0.0, 0.0]
DH_D = [0.1625, 0.0, 0.0, 0.1333, 0.0997, 0.0996]
# exact integer cos/sin of the alpha angles [pi/2, 0, 0, pi/2, -pi/2, 0]
CA = [0, 1, 1, 0, 0, 1]
SA = [1, 0, 0, 1, -1, 0]
MASS = [3.761, 8.058, 2.846, 1.37, 1.3, 0.365]
ROTOR = 0.1


class Expr:
    __slots__ = ("op", "args", "c", "id", "users", "engine", "fused_into",
                 "slot", "order")

    def __init__(self, op, args=(), c=None, i=0):
        self.op = op
        self.args = args
        self.c = c
        self.id = i
        self.users = []          # list of consumer Exprs
        self.engine = None       # 'dve' | 'act' | 'gps' | None(folded)
        self.fused_into = None   # consumer that absorbed this node
        self.slot = None
        self.order = None


class Graph:
    def __init__(self):
        self.nodes = []
        self.cse = {}

    def _mk(self, op, args=(), c=None):
        key = (op, tuple(a.id for a in args), c)
        n = self.cse.get(key)
        if n is None:
            n = Expr(op, args, c, len(self.nodes))
            self.nodes.append(n)
            self.cse[key] = n
        return n

    # ---- builders with simplification ----
    def C(self, v):
        return self._mk("const", c=float(v))

    def IN(self, ch):
        return self._mk("in", c=ch)

    def add(self, x, y):
        if x.op == "const" and y.op == "const":
            return self.C(x.c + y.c)
        if x.op == "const":
            x, y = y, x
        if y.op == "const":
            if y.c == 0.0:
                return x
            return self._mk("cadd", (x,), y.c)
        if x.op == "cmul" and x.c == -1.0:
            return self.sub(y, x.args[0])
        if y.op == "cmul" and y.c == -1.0:
            return self.sub(x, y.args[0])
        a, b = (x, y) if x.id <= y.id else (y, x)
        return self._mk("add", (a, b))

    def sub(self, x, y):
        if x.op == "const" and y.op == "const":
            return self.C(x.c - y.c)
        if y.op == "const":
            if y.c == 0.0:
                return x
            return self._mk("cadd", (x,), -y.c)
        if y.op == "cmul" and y.c == -1.0:
            return self.add(x, y.args[0])
        if x.op == "const" and x.c == 0.0:
            return self.cmul(-1.0, y)
        if x is y:
            return self.C(0.0)
        return self._mk("sub", (x, y))

    def cmul(self, c, x):
        c = float(c)
        if x.op == "const":
            return self.C(c * x.c)
        if c == 0.0:
            return self.C(0.0)
        if c == 1.0:
            return x
        if x.op == "cmul":
            return self.cmul(c * x.c, x.args[0])
        return self._mk("cmul", (x,), c)

    def mul(self, x, y):
        if x.op == "const":
            return self.cmul(x.c, y)
        if y.op == "const":
            return self.cmul(y.c, x)
        if x.op == "cmul" and y.op == "cmul":
            return self.cmul(x.c * y.c, self.mul(x.args[0], y.args[0]))
        if x.op == "cmul":
            return self.cmul(x.c, self.mul(x.args[0], y))
        if y.op == "cmul":
            return self.cmul(y.c, self.mul(x, y.args[0]))
        if x is y:
            return self._mk("square", (x,))
        a, b = (x, y) if x.id <= y.id else (y, x)
        return self._mk("mul", (a, b))

    def sinsb(self, x, scale, bias):
        # sin(scale*x + bias)
        return self._mk("sin", (x,), (float(scale), float(bias)))

    def ts2(self, x, s1, op0, s2, op1):
        # (x op0 s1) op1 s2  — one DVE tensor_scalar with two fused scalar ops
        return self._mk("ts2", (x,), (float(s1), op0, float(s2), op1))

    def trig(self, q, phase):
        """sin(q + phase) with range reduction to [-pi,pi): HW Sin LUT is
        only accurate near the principal range.  k = round((q+phase)/2pi)
        via the float magic-number trick; r0 = q - 2pi*k; sin(r0 + phase)
        with phase as activation bias."""
        MAGIC = 12582912.0  # 1.5 * 2**23: adding then subtracting rounds f32
        inv2pi = 1.0 / (2.0 * PI)
        if phase == 0.0:
            t1 = self.ts2(q, inv2pi, "mult", MAGIC, "add")
            k = self._mk("cadd", (t1,), -MAGIC)
        else:
            # phase/2pi must be added BEFORE the magic add (it would be
            # absorbed: ulp(MAGIC) = 1.0)
            t0 = self.ts2(q, inv2pi, "mult", phase * inv2pi, "add")
            t1 = self._mk("cadd", (t0,), MAGIC)
            k = self._mk("cadd", (t1,), -MAGIC)
        r0 = self.add(self.cmul(-2.0 * PI, k), q)  # fuses to one STT
        return self._mk("sin", (r0,), (1.0, float(phase)))

    def sqrt_(self, x):
        return self._mk("sqrt", (x,))

    def recip(self, x):
        return self._mk("recip", (x,))

    def dot3(self, u, v):
        t = [self.mul(u[i], v[i]) for i in range(3)]
        return self.add(self.add(t[0], t[1]), t[2])

    def cross(self, a, b):
        return [
            self.sub(self.mul(a[1], b[2]), self.mul(a[2], b[1])),
            self.sub(self.mul(a[2], b[0]), self.mul(a[0], b[2])),
            self.sub(self.mul(a[0], b[1]), self.mul(a[1], b[0])),
        ]


def build_graph():
    """Returns (graph, cost_neg_node). cost_neg = -cost per sample."""
    g = Graph()
    q = [g.IN(6 + i) for i in range(6)]
    hand = [g.IN(19 + c) for c in range(3)]
    s = [g.trig(q[i], 0.0) for i in range(6)]
    c_ = [g.trig(q[i], PI / 2) for i in range(6)]  # cos

    one, zero = g.C(1.0), g.C(0.0)
    R = [[one, zero, zero], [zero, one, zero], [zero, zero, one]]
    p = [zero, zero, zero]
    ps = [list(p)]
    zs = [[zero, zero, one]]
    for i in range(6):
        ct, st = c_[i], s[i]
        ca, sa = g.C(CA[i]), g.C(SA[i])
        # DH rotation columns
        col = [
            [ct, st, zero],
            [g.cmul(-CA[i], st) if CA[i] else zero,
             g.cmul(CA[i], ct) if CA[i] else zero, sa],
            [g.cmul(SA[i], st) if SA[i] else zero,
             g.cmul(-SA[i], ct) if SA[i] else zero, ca],
        ]
        dp = [g.cmul(DH_A[i], ct), g.cmul(DH_A[i], st), g.C(DH_D[i])]
        Rn = [[g.dot3(R[r], col[cc]) for cc in range(3)] for r in range(3)]
        pn = [g.add(p[r], g.dot3(R[r], dp)) for r in range(3)]
        R, p = Rn, pn
        ps.append(list(p))
        zs.append([R[r][2] for r in range(3)])

    # Jacobian columns J[(i,j)] = z_j x (p_{i+1} - p_j), j<=i
    J = {}
    for i in range(6):
        for j in range(i + 1):
            dif = [g.sub(ps[i + 1][cc], ps[j][cc]) for cc in range(3)]
            J[(i, j)] = g.cross(zs[j], dif)

    # mass matrix upper triangle
    M = {}
    for jj in range(6):
        for kk in range(jj, 6):
            acc = None
            for i in range(kk, 6):
                d3 = g.cmul(MASS[i], g.dot3(J[(i, jj)], J[(i, kk)]))
                acc = d3 if acc is None else g.add(acc, d3)
            if jj == kk:
                acc = g.add(acc, g.C(ROTOR))
            M[(jj, kk)] = acc

    # Cholesky M = L L^T ; keep rinv_j = 1/L_jj
    L = {}
    rinv = []
    for jc in range(6):
        dd = M[(jc, jc)]
        for t in range(jc):
            dd = g.sub(dd, g.mul(L[(jc, t)], L[(jc, t)]))
        r = g.recip(g.sqrt_(dd))
        rinv.append(r)
        for kk in range(jc + 1, 6):
            a = M[(jc, kk)]
            for t in range(jc):
                a = g.sub(a, g.mul(L[(kk, t)], L[(jc, t)]))
            L[(kk, jc)] = g.mul(a, r)

    # direction to hand, squared distance
    d = [g.sub(hand[cc], ps[6][cc]) for cc in range(3)]
    n2 = g.dot3(d, d)
    # vd = Je^T d  (Je columns are J[(5,j)])
    vd = [g.dot3(J[(5, j)], d) for j in range(6)]
    # forward solve L y = vd ; s = |y|^2
    y = []
    for j in range(6):
        a = vd[j]
        for t in range(j):
            a = g.sub(a, g.mul(L[(j, t)], y[t]))
        y.append(g.mul(a, rinv[j]))
    sacc = None
    for j in range(6):
        t = g.mul(y[j], y[j])
        sacc = t if sacc is None else g.add(sacc, t)
    # cost_neg = -n2 / s
    cost_neg = g.mul(g.cmul(-1.0, g.recip(sacc)), n2)
    return g, cost_neg


# ----------------------------------------------------------------------------
# numpy evaluation of the DAG (for validation in test.py)
# ----------------------------------------------------------------------------

def eval_numpy(g, root, chans):
    """chans: dict ch -> np array [N]. Evaluates all nodes; returns root val."""
    val = {}
    for n in g.nodes:
        if n.op == "const":
            val[n.id] = np.float32(n.c)
        elif n.op == "in":
            val[n.id] = chans[n.c]
        elif n.op == "add":
            val[n.id] = val[n.args[0].id] + val[n.args[1].id]
        elif n.op == "sub":
            val[n.id] = val[n.args[0].id] - val[n.args[1].id]
        elif n.op == "mul":
            val[n.id] = val[n.args[0].id] * val[n.args[1].id]
        elif n.op == "square":
            val[n.id] = val[n.args[0].id] * val[n.args[0].id]
        elif n.op == "cmul":
            val[n.id] = np.float32(n.c) * val[n.args[0].id]
        elif n.op == "cadd":
            val[n.id] = val[n.args[0].id] + np.float32(n.c)
        elif n.op == "sin":
            sc, b = n.c
            val[n.id] = np.sin(np.float32(sc) * val[n.args[0].id] + np.float32(b))
        elif n.op == "ts2":
            s1, op0, s2, op1 = n.c
            v = val[n.args[0].id]
            for s_, o_ in ((s1, op0), (s2, op1)):
                if o_ == "mult":
                    v = v * np.float32(s_)
                else:
                    v = v + np.float32(s_)
            val[n.id] = v
        elif n.op == "sqrt":
            val[n.id] = np.sqrt(val[n.args[0].id])
        elif n.op == "recip":
            val[n.id] = np.float32(1.0) / val[n.args[0].id]
        else:
            raise ValueError(n.op)
        if n.op != "const":
            val[n.id] = val[n.id].astype(np.float32)
    return val[root.id]


def ref_numpy(x):
    """Full-pipeline numpy reference using the DAG; x [B,H,26] -> [B]."""
    B, H, Cc = x.shape
    N = B * H
    flat = x.reshape(N, Cc).astype(np.float32)
    g, root = build_graph()
    chans = {ch: flat[:, ch] for ch in range(Cc)}
    cn = eval_numpy(g, root, chans)
    return cn.reshape(B, H).sum(axis=1)


# ----------------------------------------------------------------------------
# planning: use counts, fusion, engine assignment, slot allocation
# ----------------------------------------------------------------------------

COST = {  # ns per [128,128] f32 op, rough model for balancing
    ("dve", "tt"): 194, ("dve", "ts"): 127, ("dve", "stt"): 194,
    ("dve", "recip"): 260,
    ("act", "act"): 293,
    ("gps", "tt"): 420,
}


def plan(g, root, gps_frac=0.0):
    """Decide per-node: fusion into STT, engine, emission kind.

    Returns ordered list of nodes to emit (others folded/fused).
    """
    # use counts over live graph (reachable from root)
    reach = set()
    stack = [root]
    while stack:
        n = stack.pop()
        if n.id in reach:
            continue
        reach.add(n.id)
        stack.extend(n.args)
    for n in g.nodes:
        n.users = []
    order = [n for n in g.nodes if n.id in reach]
    for n in order:
        for a in n.args:
            a.users.append(n)

    # fusion: add/sub(x, cmul(c,y)) -> STT ; cmul(c, mul(x,y)) -> STT;
    # cmul(c, square(x)) -> STT(x,c,mult,x,mult)
    for n in order:
        if n.op in ("add", "sub"):
            for k, a in enumerate(n.args):
                if a.op == "cmul" and len(a.users) == 1 and a.fused_into is None \
                        and a.args[0].fused_into is None \
                        and a.args[0].op not in ("const",):
                    # (y*c) op other
                    n.c = ("stt_cmul", k, a.c)
                    a.fused_into = n
                    break
        elif n.op == "cmul" and n.fused_into is None:
            a = n.args[0]
            if a.op in ("mul", "square") and len(a.users) == 1 \
                    and a.fused_into is None \
                    and all(aa.fused_into is None for aa in a.args):
                # mark: n emits as STT (x*c)*y
                a.fused_into = n

    # engine assignment: greedy balance
    load = {"dve": 0.0, "act": 0.0, "gps": 0.0}
    emit = []
    for n in order:
        if n.op in ("const", "in"):
            continue
        if n.fused_into is not None:
            continue
        if n.op in ("sin", "sqrt"):
            n.engine = "act"
            load["act"] += COST[("act", "act")]
        elif n.op == "recip":
            n.engine = "dve"
            load["dve"] += COST[("dve", "recip")]
        elif n.op in ("cadd", "ts2"):
            # ACT Identity needs a registered const AP per bias value; keep on DVE
            n.engine = "dve"
            load["dve"] += COST[("dve", "ts")]
        elif n.op == "cmul" and not (isinstance(n.c, tuple)) and \
                n.args[0].fused_into is None:
            # pure affine: cheapest on DVE ts (2x mode), but ACT if idle
            if load["act"] + COST[("act", "act")] < load["dve"] + COST[("dve", "ts")]:
                n.engine = "act"
                load["act"] += COST[("act", "act")]
            else:
                n.engine = "dve"
                load["dve"] += COST[("dve", "ts")]
        elif n.op == "square":
            if load["act"] + COST[("act", "act")] < load["dve"] + COST[("dve", "tt")]:
                n.engine = "act"
                load["act"] += COST[("act", "act")]
            else:
                n.engine = "dve"
                load["dve"] += COST[("dve", "tt")]
        else:
            # tensor-tensor style (add/sub/mul/stt-fused/cmul-of-mul)
            is_stt = (n.op in ("add", "sub") and isinstance(n.c, tuple)) or \
                (n.op == "cmul" and n.args[0].fused_into is n)
            if gps_frac > 0 and not is_stt and \
                    load["gps"] + COST[("gps", "tt")] < \
                    load["dve"] + COST[("dve", "tt")]:
                n.engine = "gps"
                load["gps"] += COST[("gps", "tt")]
            else:
                n.engine = "dve"
                load["dve"] += COST[("dve", "tt")]
        emit.append(n)

    for i, n in enumerate(emit):
        n.order = i
    return emit, load


# ----------------------------------------------------------------------------
# bass emission
# ----------------------------------------------------------------------------

NCORES = 8
B_FULL, H, CH = 2048, 64, 26
N_PER_CORE = B_FULL * H // NCORES          # 16384
P = 128
FD = N_PER_CORE // P                        # 128


def _build_bass(gps_frac=0.0, repeat=1):
    import concourse.bass as bass
    from concourse.bacc import Bacc
    import concourse.mybir as mybir
    from concourse.tile import TileContext

    f32 = mybir.dt.float32
    alu = mybir.AluOpType
    AF = mybir.ActivationFunctionType

    g, root = build_graph()
    emit, load = plan(g, root, gps_frac)

    nc = Bacc()
    # register const APs needed as activation biases (non-Copy funcs)
    for cv in (PI / 2,):
        t = nc.alloc_sbuf_tensor(f"constf32-{cv}", [128, 1], f32)
        nc.gpsimd.memset(t.ap(), cv)
        nc.const_aps.aps[(f32, float(cv))] = t.ap()
    nc.all_engine_barrier()
    xs = nc.dram_tensor("xs", (N_PER_CORE, CH), f32, kind="ExternalInput")
    out = nc.dram_tensor("out", (B_FULL // NCORES,), f32, kind="ExternalOutput")

    # liveness for slot allocation
    last_use = {}
    for n in emit:
        for a in n.args:
            if a.order is not None:
                last_use[a.id] = max(last_use.get(a.id, -1), n.order)
            # fused producer's args are read by n as well
            if a.fused_into is n:
                for aa in a.args:
                    if aa.order is not None:
                        last_use[aa.id] = max(last_use.get(aa.id, -1), n.order)
    last_use[root.id] = len(emit) + 10

    with TileContext(nc) as tc:
        with tc.tile_pool(name="vals", bufs=1) as vp:
          for _rep in range(repeat):
            stage = vp.tile([P, FD * CH], f32, tag="stage", bufs=2)
            src = xs.rearrange("(p q) c -> p (q c)", p=P)
            nc.sync.dma_start(stage[:, :], src)
            stage3 = stage.rearrange("p (q c) -> p q c", c=CH)

            from collections import deque
            free_slots = deque()
            SLACK = 64  # keep reuse distance long so WAR waits are elided
            n_slots = [0]
            node_tile = {}

            def ap_of(n):
                if n.op == "in":
                    return stage3[:, :, n.c]
                return node_tile[n.id][:, :]

            def alloc(n):
                if len(free_slots) > SLACK:
                    sl = free_slots.popleft()
                else:
                    sl = n_slots[0]
                    n_slots[0] += 1
                t = vp.tile([P, FD], f32, tag=f"s{sl}", name=f"v{n.id}", bufs=2)
                n.slot = sl
                node_tile[n.id] = t
                return t

            def release_dead(i):
                for nn in emit[:0]:
                    pass

            # precompute: nodes whose last use is at order i
            by_last = {}
            for nid, lu in last_use.items():
                by_last.setdefault(lu, []).append(nid)

            eng = {"dve": nc.vector, "act": nc.scalar, "gps": nc.gpsimd}
            ALU_OF = {"add": alu.add, "sub": alu.subtract, "mul": alu.mult}

            for n in emit:
                ot = alloc(n)[:, :]
                e = eng[n.engine]
                if n.op == "sin":
                    sc, b = n.c
                    nc.scalar.activation(ot, ap_of(n.args[0]), AF.Sin,
                                         bias=float(b), scale=float(sc))
                elif n.op == "sqrt":
                    nc.scalar.activation(ot, ap_of(n.args[0]), AF.Sqrt)
                elif n.op == "recip":
                    nc.vector.reciprocal_approx_fast(out=ot, in_=ap_of(n.args[0]))
                elif n.op == "square":
                    if n.engine == "act":
                        nc.scalar.activation(ot, ap_of(n.args[0]), AF.Square)
                    else:
                        a = ap_of(n.args[0])
                        e.tensor_tensor(ot, a, a, alu.mult)
                elif n.op == "cadd":
                    if n.engine == "act":
                        nc.scalar.add(ot, ap_of(n.args[0]), float(n.c))
                    else:
                        e.tensor_scalar_add(ot, ap_of(n.args[0]), float(n.c))
                elif n.op == "ts2":
                    s1, op0, s2, op1 = n.c
                    e.tensor_scalar(ot, ap_of(n.args[0]), float(s1), float(s2),
                                    getattr(alu, op0), getattr(alu, op1))
                elif n.op == "cmul":
                    a = n.args[0]
                    if a.fused_into is n:
                        # STT: (x * c) op y
                        if a.op == "square":
                            x = yv = a.args[0]
                        else:
                            x, yv = a.args
                        e.scalar_tensor_tensor(ot, ap_of(x), float(n.c),
                                               ap_of(yv), alu.mult, alu.mult)
                    elif n.engine == "act":
                        nc.scalar.mul(ot, ap_of(n.args[0]), float(n.c))
                    else:
                        e.tensor_scalar_mul(ot, ap_of(n.args[0]), float(n.c))
                elif n.op in ("add", "sub"):
                    if isinstance(n.c, tuple) and n.c and n.c[0] == "stt_cmul":
                        _, k, cval = n.c
                        cm = n.args[k]
                        other = n.args[1 - k]
                        x = cm.args[0]
                        if n.op == "add":
                            # (x*c) + other
                            e.scalar_tensor_tensor(ot, ap_of(x), float(cval),
                                                   ap_of(other), alu.mult, alu.add)
                        else:
                            if k == 1:
                                # other - (x*c) = (x*-c) + other
                                e.scalar_tensor_tensor(ot, ap_of(x), float(-cval),
                                                       ap_of(other), alu.mult,
                                                       alu.add)
                            else:
                                # (x*c) - other
                                e.scalar_tensor_tensor(ot, ap_of(x), float(cval),
                                                       ap_of(other), alu.mult,
                                                       alu.subtract)
                    else:
                        e.tensor_tensor(ot, ap_of(n.args[0]), ap_of(n.args[1]),
                                        ALU_OF[n.op])
                elif n.op == "mul":
                    e.tensor_tensor(ot, ap_of(n.args[0]), ap_of(n.args[1]),
                                    alu.mult)
                else:
                    raise ValueError(n.op)

                # free slots whose last use was this node
                for nid in by_last.get(n.order, []):
                    nd = g.nodes[nid]
                    if nd.slot is not None and nd.id != root.id:
                        free_slots.append(nd.slot)
                        nd.slot = None

            # epilogue: per-b sums (64-sample segments), negate already folded
            osum = vp.tile([P, 2], f32, tag="osum", bufs=2)
            croot = node_tile[root.id]
            nc.vector.tensor_reduce(osum[:, 0:1], croot[:, 0:64],
                                    mybir.AxisListType.X, alu.add)
            nc.vector.tensor_reduce(osum[:, 1:2], croot[:, 64:128],
                                    mybir.AxisListType.X, alu.add)
            nc.sync.dma_start(out.rearrange("(p j) -> p j", p=P), osum[:, :])

    # run the bacc lowering passes (register allocation, wait splitting);
    # run_bass_via_pjrt serializes nc without calling finalize()
    nc.compile()
    return nc, len(emit), load, n_slots[0]


_CACHE = {}


def kernel(x, cond, time):
    from concourse.bass_utils import run_bass_kernel_spmd

    if "nc" not in _CACHE:
        import os as _os
        nc, n_ops, load, nsl = _build_bass(gps_frac=float(_os.environ.get("KERNEL_GPS", "1.0")))
        _CACHE["nc"] = nc
    nc = _CACHE["nc"]

    xf = np.ascontiguousarray(x, dtype=np.float32).reshape(B_FULL * H, CH)
    in_maps = []
    for k in range(NCORES):
        shard = xf[k * N_PER_CORE:(k + 1) * N_PER_CORE]
        in_maps.append({"xs": np.ascontiguousarray(shard)})
    res = run_bass_kernel_spmd(nc, in_maps, core_ids=list(range(NCORES)))
    _CACHE["exec_time_ns"] = res.exec_time_ns
    _CACHE["trace"] = res.instructions_and_trace
    outs = [res.results[k]["out"] for k in range(NCORES)]
    return np.concatenate(outs).astype(np.float32)


if __name__ == "__main__":
    # quick DAG stats
    g, root = build_graph()
    emit, load = plan(g, root)
    from collections import Counter
    print("emitted ops:", len(emit))
    print(Counter((n.engine, n.op) for n in emit))
    print("load est (us):", {k: v / 1000 for k, v in load.items()})
